# revision 42
# baseline (speedup 1.0000x reference)
"""Trainium2 Bass kernel for nn_EngramShortConv (RMSNorm + depthwise dilated
causal conv1d + silu), 8-core SPMD.

  x: [B=4, L=4096, HC=4, D=1024] fp32 -> y same shape/dtype.

Sharding: 16 independent (b, hc) groups, 2 per NeuronCore, zero communication.

v3:
  - Output written CHANNEL-MAJOR straight from conv/silu PSUM; host does the
    final un-transpose during unpack (deletes the old PE transpose-back pass
    and its PSUM->SBUF copies).
  - All HBM layouts host-swizzled so every DMA is 128 partitions x 8KB
    contiguous (128 descriptors instead of 512-1024 small ones).
  - Conv diag stationaries built on device from compact per-channel weight
    columns (kills a 2MB / 8192-descriptor weight load).
  - Stats (squares -> rsqrt -> diag(r)) run per *chunk* and are emitted one
    pair ahead so the PE never waits on them.

Per core, per 512-token chunk:
  1. stats: x^2 with 1/D folded accumulates to ms per token (engine per
     128-token block set by SQ_ENGINES); r = rsqrt(ms+eps) via bit-trick +
     1 Newton step on DVE (no ACT table swaps).
  2. pass1 (PE): Z[d, t] = X_blk^T @ diag(r) per 128x128 block -- transpose
     to channel-major with the RMSNorm scale folded in. DVE/ACT copy
     PSUM -> SBUF fp16 (two slabs per instruction) with a 6-column halo
     from the previous chunk.
  3. pass2 (PE): depthwise conv as 4 PSUM-accumulated matmuls
     diag(conv_w[k] * norm_w) @ Z[:, t - 6 + 2k].
  4. ACT Silu reads conv PSUM -> fp16 SBUF; DMA out channel-major.

I/O precision: host casts x to fp16 (halves input DMA); device returns fp16
y upcast to fp32 on host. End-to-end scale-relative error ~3e-3.
"""

import sys

if "/opt/trn_rl_repo" not in sys.path:
    sys.path.insert(0, "/opt/trn_rl_repo")

import numpy as np

B, L, HC, D = 4, 4096, 4, 1024
K, DIL = 4, 2
EPS = 1e-5
PAD = (K - 1) * DIL  # 6
NCORES = 8
NGROUPS = B * HC     # 16
GPC = NGROUPS // NCORES  # 2 groups per core

# tunables
TCH = 512            # tokens per chunk (= matmul moving free dim)
CPAIR = 2            # chunks per conv pairing (shares conv ldweights)
SQ_ENGINES = ("vector", "act", "vector", "vector")  # even chunks (odd use a
# different DVE/ACT interleave so consecutive chunks' stats can't collide on
# one engine queue; see odd_engines below)
ZCOPY_ACT = 1        # of 4 two-slab zcopy units per chunk, how many on ACT

_prog_cache = {}


def build_program(L_=L, gpc=GPC, tch=TCH, cpair=CPAIR,
                  sq_engines=SQ_ENGINES, zcopy_act=ZCOPY_ACT):
    """Build the per-core Bacc program. Same program on all cores (SPMD)."""
    import concourse.bacc as bacc
    import concourse.tile as tile
    from concourse import mybir

    f32 = mybir.dt.float32
    f16 = mybir.dt.float16
    i32 = mybir.dt.int32
    AF = mybir.ActivationFunctionType
    ALU = mybir.AluOpType

    nblk = tch // 128
    dsub = D // 128
    nchunks = L_ // tch
    assert tch % 128 == 0 and L_ % tch == 0 and D % 128 == 0

    nc = bacc.Bacc()
    # host-swizzled input: (g, c, p, blk, d) = x[g, c*tch + blk*128 + p, d]
    xin = nc.declare_dram_parameter("xin", [gpc, nchunks, 128, nblk, D], f16,
                                    isOutput=False)
    # host-built diag stationaries, partition-major so the DMA is 128
    # contiguous 16KB rows: (p, g, k, s, m) = diag(w_eff)[g, k, s][p, m]
    wdg = nc.declare_dram_parameter("wdg", [128, gpc, K, dsub, 128], f16,
                                    isOutput=False)
    idn = nc.declare_dram_parameter("idn", [128, 128], f16, isOutput=False)
    # channel-major output: (g, c, p, s, t) = y[g, d=s*128+p, l=c*tch+t]
    yout = nc.declare_dram_parameter("yout", [gpc, nchunks, 128, dsub, tch],
                                     f16, isOutput=True)

    xv = xin[:]
    yv = yout[:]

    with tile.TileContext(nc) as tc:
        with (
            tc.tile_pool(name="pconst", bufs=1) as pconst,
            tc.tile_pool(name="px", bufs=10) as px,
            tc.tile_pool(name="pstat", bufs=4) as pstat,
            tc.tile_pool(name="pz", bufs=5) as pz,
            tc.tile_pool(name="py", bufs=4) as py,
            tc.tile_pool(name="pp1", bufs=2, space="PSUM") as pp1,
            tc.tile_pool(name="pp2", bufs=2, space="PSUM") as pp2,
        ):
            ident = pconst.tile([128, 128], f16)
            nc.scalar.dma_start(out=ident[:], in_=idn[:])
            wsb = pconst.tile([128, gpc, K, dsub, 128], f16)
            nc.scalar.dma_start(out=wsb[:], in_=wdg[:])

            def emit_stats(g, c, b0, nb):
                """Load a (sub-)chunk of nb 128-token blocks + stats + r +
                drt, emitted pieces ahead so diag(r) never gates PE."""
                xh = px.tile([128, nblk, D], f16, tag="xh")
                nc.sync.dma_start(out=xh[:, 0:nb, :],
                                  in_=xv[g, c][:, b0:b0 + nb, :])

                ssq = pstat.tile([128, nblk], f32, tag="ssq")
                odd_engines = ("act", "vector", "vector", "act")
                for blk in range(nb):
                    if nb < nblk:
                        eng = "vector"
                    else:
                        eng = (sq_engines if c % 2 == 0
                               else odd_engines)[blk % len(sq_engines)]
                    scr = pstat.tile([128, D], f16, tag="scr")
                    if eng == "act":
                        nc.scalar.activation(
                            out=scr[:], in_=xh[:, blk, :],
                            func=AF.Square, scale=float(D) ** -0.5,
                            accum_out=ssq[:, blk:blk + 1])
                    else:
                        nc.vector.scalar_tensor_tensor(
                            out=scr[:], in0=xh[:, blk, :],
                            scalar=1.0 / D, in1=xh[:, blk, :],
                            op0=ALU.mult, op1=ALU.mult,
                            accum_out=ssq[:, blk:blk + 1])
                # r = rsqrt(ms+eps): bit trick + 1 Newton step on
                # DVE (keeps Sqrt out of ACT -> zero table swaps)
                v = pstat.tile([128, nblk], f32, tag="v")
                nc.vector.tensor_scalar(
                    out=v[:, 0:nb], in0=ssq[:, 0:nb], scalar1=EPS,
                    scalar2=None, op0=ALU.add)
                r = pstat.tile([128, nblk], f32, tag="r")
                nc.vector.tensor_scalar(
                    out=r[:, 0:nb].bitcast(i32), in0=v[:, 0:nb].bitcast(i32),
                    scalar1=1, scalar2=None, op0=ALU.arith_shift_right)
                nc.vector.tensor_scalar(
                    out=r[:, 0:nb].bitcast(i32), in0=r[:, 0:nb].bitcast(i32),
                    scalar1=-1, scalar2=0x5F3759DF,
                    op0=ALU.mult, op1=ALU.add)
                yy = pstat.tile([128, nblk], f32, tag="yy")
                nc.vector.tensor_tensor(
                    out=yy[:, 0:nb], in0=r[:, 0:nb], in1=r[:, 0:nb],
                    op=ALU.mult)
                nc.vector.tensor_tensor(
                    out=yy[:, 0:nb], in0=yy[:, 0:nb], in1=v[:, 0:nb],
                    op=ALU.mult)
                nc.vector.tensor_scalar(
                    out=yy[:, 0:nb], in0=yy[:, 0:nb], scalar1=-0.5,
                    scalar2=1.5, op0=ALU.mult, op1=ALU.add)
                rn = pstat.tile([128, nblk], f32, tag="rn")
                nc.vector.tensor_tensor(
                    out=rn[:, 0:nb], in0=r[:, 0:nb], in1=yy[:, 0:nb],
                    op=ALU.mult)

                drt = pstat.tile([128, nblk, 128], f16, tag="drt")
                for blk in range(nb):
                    nc.vector.tensor_scalar_mul(
                        out=drt[:, blk, :], in0=ident[:],
                        scalar1=rn[:, blk:blk + 1])
                return (xh, drt)

            # piece schedule: the first chunk is split 128/128/256 so the
            # PE starts as soon as the first 128 tokens' stats are done; the
            # last chunk is split 256/256 to shorten the drain.
            pieces = []
            for g in range(gpc):
                for c in range(nchunks):
                    if g == 0 and c == 0:
                        pieces += [(g, c, 0, 1), (g, c, 1, 1), (g, c, 2, 2)]
                    elif g == gpc - 1 and c == nchunks - 1:
                        pieces += [(g, c, 0, 2), (g, c, 2, 2)]
                    else:
                        pieces.append((g, c, 0, nblk))

            zt_prev = None
            zt_prev_len = 0
            pending = [emit_stats(*pieces[0]), emit_stats(*pieces[1])]
            for pidx, (g, c, b0, nb) in enumerate(pieces):
                xh, drt = pending.pop(0)
                if pidx + 2 < len(pieces):
                    pending.append(emit_stats(*pieces[pidx + 2]))
                plen = nb * 128

                # ---- pass1: Z[d, t] = X^T diag(r) ----
                zt = pz.tile([128, dsub, PAD + tch], f16, tag="zt")
                if c == 0 and b0 == 0:
                    nc.vector.memset(zt[:, :, 0:PAD], 0.0)
                else:
                    nc.vector.tensor_copy(
                        out=zt[:, :, 0:PAD],
                        in_=zt_prev[:, :, zt_prev_len:zt_prev_len + PAD])
                for si in range(dsub // 2):
                    zp = pp1.tile([128, 2, tch], f32, tag="zp")
                    for sh in range(2):
                        s = 2 * si + sh
                        for blk in range(nb):
                            nc.tensor.matmul(
                                zp[:, sh, blk * 128:(blk + 1) * 128],
                                lhsT=xh[:, blk, s * 128:(s + 1) * 128],
                                rhs=drt[:, blk, :],
                                start=True, stop=True)
                    dst = zt[:, 2 * si:2 * si + 2, PAD:PAD + plen]
                    if si < zcopy_act:
                        nc.scalar.copy(out=dst, in_=zp[:, :, 0:plen])
                    else:
                        nc.vector.tensor_copy(out=dst, in_=zp[:, :, 0:plen])
                zt_prev = zt
                zt_prev_len = plen

                # ---- pass2: conv + silu + store ----
                yh = py.tile([128, dsub, tch], f16, tag="yh")
                for si in range(dsub // 2):
                    yp = pp2.tile([128, 2, tch], f32, tag="yp")
                    for sh in range(2):
                        s = 2 * si + sh
                        for k in range(K):
                            nc.tensor.matmul(
                                yp[:, sh, 0:plen],
                                lhsT=wsb[:, g, k, s, :],
                                rhs=zt[:, s, k * DIL:k * DIL + plen],
                                start=(k == 0), stop=(k == K - 1))
                    nc.scalar.activation(
                        out=yh[:, 2 * si:2 * si + 2, 0:plen],
                        in_=yp[:, :, 0:plen], func=AF.Silu)
                    if nb == nblk and si == dsub // 4 - 1:
                        nc.gpsimd.dma_start(
                            out=yv[g, c, :, 0:dsub // 2],
                            in_=yh[:, 0:dsub // 2])
                if nb == nblk:
                    nc.gpsimd.dma_start(out=yv[g, c, :, dsub // 2:dsub],
                                        in_=yh[:, dsub // 2:dsub])
                else:
                    nc.gpsimd.dma_start(
                        out=yv[g, c][:, :, b0 * 128:b0 * 128 + plen],
                        in_=yh[:, :, 0:plen])
    nc.compile()
    return nc


def _host_pack(x, norm_weight, conv_weight):
    """Shard inputs across cores; swizzle for contiguous DMA; fold norm
    weight into compact per-channel conv weight columns."""
    dsub = D // 128
    nblk = TCH // 128
    nchunks = L // TCH
    xg = np.ascontiguousarray(x.transpose(0, 2, 1, 3)).reshape(NGROUPS, L, D)
    xg = xg.astype(np.float16)
    # (g, c, p, blk, d) = x[g, c*tch + blk*128 + p, d]
    xsw = np.ascontiguousarray(
        xg.reshape(NGROUPS, nchunks, nblk, 128, D).transpose(0, 1, 3, 2, 4))

    conv_w = conv_weight.reshape(HC, D, K)            # [hc, d, k]
    weff = conv_w * norm_weight[:, :, None]           # [hc, d, k]
    wr = weff.transpose(0, 2, 1).reshape(HC, K, dsub, 128)  # [hc, k, s, p]
    eye = np.eye(128, dtype=np.float32)
    wdiag = (wr[..., None] * eye).astype(np.float16)  # [hc, K, s, p, m]
    idn = np.eye(128, dtype=np.float16)

    in_maps = []
    for i in range(NCORES):
        gs = [i * GPC + j for j in range(GPC)]
        wcore = np.stack([wdiag[g % HC] for g in gs])  # [gpc, K, s, p, m]
        wpm = np.ascontiguousarray(
            wcore.transpose(3, 0, 1, 2, 4))            # [p, gpc, K, s, m]
        in_maps.append({
            "xin": np.ascontiguousarray(xsw[gs[0]:gs[-1] + 1]),
            "wdg": wpm,
            "idn": idn,
        })
    return in_maps


def _host_unpack(results):
    dsub = D // 128
    nchunks = L // TCH
    # yout per core: [gpc, nchunks, 128, dsub, tch] channel-major
    ys = np.concatenate([r["yout"] for r in results], axis=0)
    ys = ys.reshape(B, HC, nchunks, 128, dsub, TCH)
    # [b, hc, c, p, s, t] -> [b, (c t), hc, (s p)]
    y = ys.transpose(0, 2, 5, 1, 4, 3).reshape(B, L, HC, D)
    return np.ascontiguousarray(y.astype(np.float32))


def _get_prog():
    key = (L, GPC, TCH, CPAIR, SQ_ENGINES, ZCOPY_ACT)
    if key not in _prog_cache:
        _prog_cache[key] = build_program()
    return _prog_cache[key]


def kernel(x, norm_weight, conv_weight, _trace=False, _trace_kwargs=None):
    from concourse.bass_utils import run_bass_kernel_spmd

    x = np.asarray(x, dtype=np.float32)
    norm_weight = np.asarray(norm_weight, dtype=np.float32)
    conv_weight = np.asarray(conv_weight, dtype=np.float32)

    nc = _get_prog()
    in_maps = _host_pack(x, norm_weight, conv_weight)
    res = run_bass_kernel_spmd(
        nc, in_maps, list(range(NCORES)),
        trace=_trace, **(_trace_kwargs or {}))
    out = _host_unpack(res.results)
    if _trace:
        return out, res
    return out


# revision 43
# speedup vs baseline: 1.1062x; 1.1062x over previous
"""Trainium2 Bass kernel for nn_EngramShortConv (RMSNorm + depthwise dilated
causal conv1d + silu), 8-core SPMD.

  x: [B=4, L=4096, HC=4, D=1024] fp32 -> y same shape/dtype.

Sharding: 16 independent (b, hc) groups, 2 per NeuronCore, zero communication.

v3:
  - Output written CHANNEL-MAJOR straight from conv/silu PSUM; host does the
    final un-transpose during unpack (deletes the old PE transpose-back pass
    and its PSUM->SBUF copies).
  - All HBM layouts host-swizzled so every DMA is 128 partitions x 8KB
    contiguous (128 descriptors instead of 512-1024 small ones).
  - Conv diag stationaries built on device from compact per-channel weight
    columns (kills a 2MB / 8192-descriptor weight load).
  - Stats (squares -> rsqrt -> diag(r)) run per *chunk* and are emitted one
    pair ahead so the PE never waits on them.

Per core, per 512-token chunk:
  1. stats: x^2 with 1/D folded accumulates to ms per token (engine per
     128-token block set by SQ_ENGINES); r = rsqrt(ms+eps) via bit-trick +
     1 Newton step on DVE (no ACT table swaps).
  2. pass1 (PE): Z[d, t] = X_blk^T @ diag(r) per 128x128 block -- transpose
     to channel-major with the RMSNorm scale folded in. DVE/ACT copy
     PSUM -> SBUF fp16 (two slabs per instruction) with a 6-column halo
     from the previous chunk.
  3. pass2 (PE): depthwise conv as 4 PSUM-accumulated matmuls
     diag(conv_w[k] * norm_w) @ Z[:, t - 6 + 2k].
  4. ACT Silu reads conv PSUM -> fp16 SBUF; DMA out channel-major.

I/O precision: host casts x to fp16 (halves input DMA); device returns fp16
y upcast to fp32 on host. End-to-end scale-relative error ~3e-3.
"""

import sys

if "/opt/trn_rl_repo" not in sys.path:
    sys.path.insert(0, "/opt/trn_rl_repo")

import numpy as np

B, L, HC, D = 4, 4096, 4, 1024
K, DIL = 4, 2
EPS = 1e-5
PAD = (K - 1) * DIL  # 6
NCORES = 8
NGROUPS = B * HC     # 16
GPC = NGROUPS // NCORES  # 2 groups per core

# tunables
TCH = 512            # tokens per chunk (= matmul moving free dim)
CPAIR = 2            # chunks per conv pairing (shares conv ldweights)
SQ_ENGINES = ("vector", "act", "vector", "vector")  # even chunks (odd use a
# different DVE/ACT interleave so consecutive chunks' stats can't collide on
# one engine queue; see odd_engines below)
ZCOPY_ACT = 1        # of 4 two-slab zcopy units per chunk, how many on ACT

_prog_cache = {}


def build_program(L_=L, gpc=GPC, tch=TCH, cpair=CPAIR,
                  sq_engines=SQ_ENGINES, zcopy_act=ZCOPY_ACT):
    """Build the per-core Bacc program. Same program on all cores (SPMD)."""
    import concourse.bacc as bacc
    import concourse.tile as tile
    from concourse import mybir

    f32 = mybir.dt.float32
    f16 = mybir.dt.float16
    i32 = mybir.dt.int32
    AF = mybir.ActivationFunctionType
    ALU = mybir.AluOpType

    nblk = tch // 128
    dsub = D // 128
    nchunks = L_ // tch
    assert tch % 128 == 0 and L_ % tch == 0 and D % 128 == 0

    nc = bacc.Bacc()
    # host-swizzled input: (g, c, p, blk, d) = x[g, c*tch + blk*128 + p, d]
    xin = nc.declare_dram_parameter("xin", [gpc, nchunks, 128, nblk, D], f16,
                                    isOutput=False)
    # host-built diag stationaries, partition-major so the DMA is 128
    # contiguous 16KB rows: (p, g, k, s, m) = diag(w_eff)[g, k, s][p, m]
    wdg = nc.declare_dram_parameter("wdg", [128, gpc, K, dsub, 128], f16,
                                    isOutput=False)
    idn = nc.declare_dram_parameter("idn", [128, 128], f16, isOutput=False)
    # channel-major output: (g, c, p, s, t) = y[g, d=s*128+p, l=c*tch+t]
    yout = nc.declare_dram_parameter("yout", [gpc, nchunks, 128, dsub, tch],
                                     f16, isOutput=True)

    xv = xin[:]
    yv = yout[:]

    with tile.TileContext(nc) as tc:
        with (
            tc.tile_pool(name="pconst", bufs=1) as pconst,
            tc.tile_pool(name="px", bufs=10) as px,
            tc.tile_pool(name="pstat", bufs=4) as pstat,
            tc.tile_pool(name="pz", bufs=5) as pz,
            tc.tile_pool(name="py", bufs=4) as py,
            tc.tile_pool(name="pp1", bufs=2, space="PSUM") as pp1,
            tc.tile_pool(name="pp2", bufs=2, space="PSUM") as pp2,
        ):
            ident = pconst.tile([128, 128], f16)
            nc.scalar.dma_start(out=ident[:], in_=idn[:])
            wsb = pconst.tile([128, gpc, K, dsub, 128], f16)
            nc.scalar.dma_start(out=wsb[:], in_=wdg[:])

            def emit_stats(g, c):
                """Load one chunk + stats + r + drt, emitted a pair ahead of
                the heavy compute so diag(r) never gates PE."""
                xh = px.tile([128, nblk, D], f16, tag="xh")
                nc.sync.dma_start(out=xh[:], in_=xv[g, c])

                ssq = pstat.tile([128, nblk], f32, tag="ssq")
                odd_engines = ("act", "vector", "vector", "act")
                for blk in range(nblk):
                    eng = (sq_engines if c % 2 == 0
                           else odd_engines)[blk % len(sq_engines)]
                    scr = pstat.tile([128, D], f16, tag="scr")
                    if eng == "act":
                        nc.scalar.activation(
                            out=scr[:], in_=xh[:, blk, :],
                            func=AF.Square, scale=float(D) ** -0.5,
                            accum_out=ssq[:, blk:blk + 1])
                    else:
                        nc.vector.scalar_tensor_tensor(
                            out=scr[:], in0=xh[:, blk, :],
                            scalar=1.0 / D, in1=xh[:, blk, :],
                            op0=ALU.mult, op1=ALU.mult,
                            accum_out=ssq[:, blk:blk + 1])
                # r = rsqrt(ms+eps): bit trick + 1 Newton step on
                # DVE (keeps Sqrt out of ACT -> zero table swaps)
                v = pstat.tile([128, nblk], f32, tag="v")
                nc.vector.tensor_scalar(
                    out=v[:], in0=ssq[:], scalar1=EPS, scalar2=None,
                    op0=ALU.add)
                r = pstat.tile([128, nblk], f32, tag="r")
                nc.vector.tensor_scalar(
                    out=r[:].bitcast(i32), in0=v[:].bitcast(i32),
                    scalar1=1, scalar2=None, op0=ALU.arith_shift_right)
                nc.vector.tensor_scalar(
                    out=r[:].bitcast(i32), in0=r[:].bitcast(i32),
                    scalar1=-1, scalar2=0x5F3759DF,
                    op0=ALU.mult, op1=ALU.add)
                yy = pstat.tile([128, nblk], f32, tag="yy")
                nc.vector.tensor_tensor(
                    out=yy[:], in0=r[:], in1=r[:], op=ALU.mult)
                nc.vector.tensor_tensor(
                    out=yy[:], in0=yy[:], in1=v[:], op=ALU.mult)
                nc.vector.tensor_scalar(
                    out=yy[:], in0=yy[:], scalar1=-0.5, scalar2=1.5,
                    op0=ALU.mult, op1=ALU.add)
                rn = pstat.tile([128, nblk], f32, tag="rn")
                nc.vector.tensor_tensor(
                    out=rn[:], in0=r[:], in1=yy[:], op=ALU.mult)

                drt = pstat.tile([128, nblk, 128], f16, tag="drt")
                for blk in range(nblk):
                    nc.vector.tensor_scalar_mul(
                        out=drt[:, blk, :], in0=ident[:],
                        scalar1=rn[:, blk:blk + 1])
                return (xh, drt)

            zt_prev = None
            pair_keys = [(g, c0) for g in range(gpc)
                         for c0 in range(0, nchunks, cpair)]
            pending = [emit_stats(pair_keys[0][0], pair_keys[0][1] + j)
                       for j in range(cpair)]
            for pidx, (g, c0) in enumerate(pair_keys):
                cs = list(range(c0, min(c0 + cpair, nchunks)))
                ncs = len(cs)
                cur = pending
                if pidx + 1 < len(pair_keys):
                    g2, c2 = pair_keys[pidx + 1]
                    pending = [emit_stats(g2, c2 + j) for j in range(cpair)]

                # ---- pass1 per chunk: Z[d, t] = X^T diag(r) ----
                zts = []
                for j, c in enumerate(cs):
                    xh, drt = cur[j]
                    zt = pz.tile([128, dsub, PAD + tch], f16, tag="zt")
                    if c == 0:
                        nc.vector.memset(zt[:, :, 0:PAD], 0.0)
                    else:
                        nc.vector.tensor_copy(
                            out=zt[:, :, 0:PAD],
                            in_=zt_prev[:, :, tch:tch + PAD])
                    for si in range(dsub // 2):
                        zp = pp1.tile([128, 2, tch], f32, tag="zp")
                        for sh in range(2):
                            s = 2 * si + sh
                            for blk in range(nblk):
                                nc.tensor.matmul(
                                    zp[:, sh, blk * 128:(blk + 1) * 128],
                                    lhsT=xh[:, blk, s * 128:(s + 1) * 128],
                                    rhs=drt[:, blk, :],
                                    start=True, stop=True)
                        dst = zt[:, 2 * si:2 * si + 2, PAD:PAD + tch]
                        if si < zcopy_act:
                            nc.scalar.copy(out=dst, in_=zp[:])
                        else:
                            nc.vector.tensor_copy(out=dst, in_=zp[:])
                    zt_prev = zt
                    zts.append(zt)

                # ---- pass2 paired: conv matmuls share ldweights;
                #      silu writes fp16 channel-major, DMA straight out
                for j, c in enumerate(cs):
                    yh = py.tile([128, dsub, tch], f16, tag="yh")
                    for si in range(dsub // 2):
                        yp = pp2.tile([128, 2, tch], f32, tag="yp")
                        for sh in range(2):
                            s = 2 * si + sh
                            for k in range(K):
                                nc.tensor.matmul(
                                    yp[:, sh, :],
                                    lhsT=wsb[:, g, k, s, :],
                                    rhs=zts[j][:, s, k * DIL:k * DIL + tch],
                                    start=(k == 0), stop=(k == K - 1))
                        nc.scalar.activation(
                            out=yh[:, 2 * si:2 * si + 2, :],
                            in_=yp[:], func=AF.Silu)
                        if si == dsub // 4 - 1:
                            nc.gpsimd.dma_start(
                                out=yv[g, c, :, 0:dsub // 2],
                                in_=yh[:, 0:dsub // 2])
                    nc.gpsimd.dma_start(out=yv[g, c, :, dsub // 2:dsub],
                                        in_=yh[:, dsub // 2:dsub])
    nc.compile()
    return nc


def _host_pack(x, norm_weight, conv_weight):
    """Shard inputs across cores; swizzle for contiguous DMA; fold norm
    weight into compact per-channel conv weight columns."""
    dsub = D // 128
    nblk = TCH // 128
    nchunks = L // TCH
    xg = np.ascontiguousarray(x.transpose(0, 2, 1, 3)).reshape(NGROUPS, L, D)
    xg = xg.astype(np.float16)
    # (g, c, p, blk, d) = x[g, c*tch + blk*128 + p, d]
    xsw = np.ascontiguousarray(
        xg.reshape(NGROUPS, nchunks, nblk, 128, D).transpose(0, 1, 3, 2, 4))

    conv_w = conv_weight.reshape(HC, D, K)            # [hc, d, k]
    weff = conv_w * norm_weight[:, :, None]           # [hc, d, k]
    wr = weff.transpose(0, 2, 1).reshape(HC, K, dsub, 128)  # [hc, k, s, p]
    eye = np.eye(128, dtype=np.float32)
    wdiag = (wr[..., None] * eye).astype(np.float16)  # [hc, K, s, p, m]
    idn = np.eye(128, dtype=np.float16)

    in_maps = []
    for i in range(NCORES):
        gs = [i * GPC + j for j in range(GPC)]
        wcore = np.stack([wdiag[g % HC] for g in gs])  # [gpc, K, s, p, m]
        wpm = np.ascontiguousarray(
            wcore.transpose(3, 0, 1, 2, 4))            # [p, gpc, K, s, m]
        in_maps.append({
            "xin": np.ascontiguousarray(xsw[gs[0]:gs[-1] + 1]),
            "wdg": wpm,
            "idn": idn,
        })
    return in_maps


def _host_unpack(results):
    dsub = D // 128
    nchunks = L // TCH
    # yout per core: [gpc, nchunks, 128, dsub, tch] channel-major
    ys = np.concatenate([r["yout"] for r in results], axis=0)
    ys = ys.reshape(B, HC, nchunks, 128, dsub, TCH)
    # [b, hc, c, p, s, t] -> [b, (c t), hc, (s p)]
    y = ys.transpose(0, 2, 5, 1, 4, 3).reshape(B, L, HC, D)
    return np.ascontiguousarray(y.astype(np.float32))


def _get_prog():
    key = (L, GPC, TCH, CPAIR, SQ_ENGINES, ZCOPY_ACT)
    if key not in _prog_cache:
        _prog_cache[key] = build_program()
    return _prog_cache[key]


def kernel(x, norm_weight, conv_weight, _trace=False, _trace_kwargs=None):
    from concourse.bass_utils import run_bass_kernel_spmd

    x = np.asarray(x, dtype=np.float32)
    norm_weight = np.asarray(norm_weight, dtype=np.float32)
    conv_weight = np.asarray(conv_weight, dtype=np.float32)

    nc = _get_prog()
    in_maps = _host_pack(x, norm_weight, conv_weight)
    res = run_bass_kernel_spmd(
        nc, in_maps, list(range(NCORES)),
        trace=_trace, **(_trace_kwargs or {}))
    out = _host_unpack(res.results)
    if _trace:
        return out, res
    return out


# revision 46
# speedup vs baseline: 1.1122x; 1.0053x over previous
"""Trainium2 Bass kernel for nn_EngramShortConv (RMSNorm + depthwise dilated
causal conv1d + silu), 8-core SPMD.

  x: [B=4, L=4096, HC=4, D=1024] fp32 -> y same shape/dtype.

Sharding: 16 independent (b, hc) groups, 2 per NeuronCore, zero communication.

v3:
  - Output written CHANNEL-MAJOR straight from conv/silu PSUM; host does the
    final un-transpose during unpack (deletes the old PE transpose-back pass
    and its PSUM->SBUF copies).
  - All HBM layouts host-swizzled so every DMA is 128 partitions x 8KB
    contiguous (128 descriptors instead of 512-1024 small ones).
  - Conv diag stationaries built on device from compact per-channel weight
    columns (kills a 2MB / 8192-descriptor weight load).
  - Stats (squares -> rsqrt -> diag(r)) run per *chunk* and are emitted one
    pair ahead so the PE never waits on them.

Per core, per 512-token chunk:
  1. stats: x^2 with 1/D folded accumulates to ms per token (engine per
     128-token block set by SQ_ENGINES); r = rsqrt(ms+eps) via bit-trick +
     1 Newton step on DVE (no ACT table swaps).
  2. pass1 (PE): Z[d, t] = X_blk^T @ diag(r) per 128x128 block -- transpose
     to channel-major with the RMSNorm scale folded in. DVE/ACT copy
     PSUM -> SBUF fp16 (two slabs per instruction) with a 6-column halo
     from the previous chunk.
  3. pass2 (PE): depthwise conv as 4 PSUM-accumulated matmuls
     diag(conv_w[k] * norm_w) @ Z[:, t - 6 + 2k].
  4. ACT Silu reads conv PSUM -> fp16 SBUF; DMA out channel-major.

I/O precision: host casts x to fp16 (halves input DMA); device returns fp16
y upcast to fp32 on host. End-to-end scale-relative error ~3e-3.
"""

import sys

if "/opt/trn_rl_repo" not in sys.path:
    sys.path.insert(0, "/opt/trn_rl_repo")

import numpy as np

B, L, HC, D = 4, 4096, 4, 1024
K, DIL = 4, 2
EPS = 1e-5
PAD = (K - 1) * DIL  # 6
NCORES = 8
NGROUPS = B * HC     # 16
GPC = NGROUPS // NCORES  # 2 groups per core

# tunables
TCH = 512            # tokens per chunk (= matmul moving free dim)
CPAIR = 2            # chunks per conv pairing (shares conv ldweights)
SQ_ENGINES = ("vector", "act", "vector", "vector")  # even chunks (odd use a
# different DVE/ACT interleave so consecutive chunks' stats can't collide on
# one engine queue; see odd_engines below)
ZCOPY_ACT = 1        # of 4 two-slab zcopy units per chunk, how many on ACT

_prog_cache = {}


def build_program(L_=L, gpc=GPC, tch=TCH, cpair=CPAIR,
                  sq_engines=SQ_ENGINES, zcopy_act=ZCOPY_ACT):
    """Build the per-core Bacc program. Same program on all cores (SPMD)."""
    import concourse.bacc as bacc
    import concourse.tile as tile
    from concourse import mybir

    f32 = mybir.dt.float32
    f16 = mybir.dt.float16
    i32 = mybir.dt.int32
    AF = mybir.ActivationFunctionType
    ALU = mybir.AluOpType

    nblk = tch // 128
    dsub = D // 128
    nchunks = L_ // tch
    assert tch % 128 == 0 and L_ % tch == 0 and D % 128 == 0

    nc = bacc.Bacc()
    # host-swizzled input: (g, c, p, blk, d) = x[g, c*tch + blk*128 + p, d]
    xin = nc.declare_dram_parameter("xin", [gpc, nchunks, 128, nblk, D], f16,
                                    isOutput=False)
    # host-built diag stationaries, partition-major so the DMA is 128
    # contiguous 16KB rows: (p, g, k, s, m) = diag(w_eff)[g, k, s][p, m]
    wdg = nc.declare_dram_parameter("wdg", [128, gpc, K, dsub, 128], f16,
                                    isOutput=False)
    idn = nc.declare_dram_parameter("idn", [128, 128], f16, isOutput=False)
    # channel-major output: (g, c, p, s, t) = y[g, d=s*128+p, l=c*tch+t]
    yout = nc.declare_dram_parameter("yout", [gpc, nchunks, 128, dsub, tch],
                                     f16, isOutput=True)

    xv = xin[:]
    yv = yout[:]

    with tile.TileContext(nc) as tc:
        with (
            tc.tile_pool(name="pconst", bufs=1) as pconst,
            tc.tile_pool(name="px", bufs=10) as px,
            tc.tile_pool(name="pstat", bufs=4) as pstat,
            tc.tile_pool(name="pz", bufs=5) as pz,
            tc.tile_pool(name="py", bufs=4) as py,
            tc.tile_pool(name="pp1", bufs=2, space="PSUM") as pp1,
            tc.tile_pool(name="pp2", bufs=2, space="PSUM") as pp2,
        ):
            ident = pconst.tile([128, 128], f16)
            nc.scalar.dma_start(out=ident[:], in_=idn[:])
            wsb = pconst.tile([128, gpc, K, dsub, 128], f16)
            nc.scalar.dma_start(out=wsb[:], in_=wdg[:])

            def emit_load(g, c):
                """Chunk DMA, emitted early to keep prefetch depth."""
                xh = px.tile([128, nblk, D], f16, tag="xh")
                nc.sync.dma_start(out=xh[:], in_=xv[g, c])
                return xh

            def emit_stats(g, c, xh):
                """Stats + r + drt for one chunk. Emitted AFTER the previous
                pair's zcopies so those never queue behind these DVE ops."""
                ssq = pstat.tile([128, nblk], f32, tag="ssq")
                odd_engines = ("act", "vector", "vector", "act")
                for blk in range(nblk):
                    eng = (sq_engines if c % 2 == 0
                           else odd_engines)[blk % len(sq_engines)]
                    scr = pstat.tile([128, D], f16, tag="scr")
                    if eng == "act":
                        nc.scalar.activation(
                            out=scr[:], in_=xh[:, blk, :],
                            func=AF.Square, scale=float(D) ** -0.5,
                            accum_out=ssq[:, blk:blk + 1])
                    else:
                        nc.vector.scalar_tensor_tensor(
                            out=scr[:], in0=xh[:, blk, :],
                            scalar=1.0 / D, in1=xh[:, blk, :],
                            op0=ALU.mult, op1=ALU.mult,
                            accum_out=ssq[:, blk:blk + 1])
                # r = rsqrt(ms+eps): bit trick + 1 Newton step on
                # DVE (keeps Sqrt out of ACT -> zero table swaps)
                v = pstat.tile([128, nblk], f32, tag="v")
                nc.vector.tensor_scalar(
                    out=v[:], in0=ssq[:], scalar1=EPS, scalar2=None,
                    op0=ALU.add)
                r = pstat.tile([128, nblk], f32, tag="r")
                nc.vector.tensor_scalar(
                    out=r[:].bitcast(i32), in0=v[:].bitcast(i32),
                    scalar1=1, scalar2=None, op0=ALU.arith_shift_right)
                nc.vector.tensor_scalar(
                    out=r[:].bitcast(i32), in0=r[:].bitcast(i32),
                    scalar1=-1, scalar2=0x5F3759DF,
                    op0=ALU.mult, op1=ALU.add)
                yy = pstat.tile([128, nblk], f32, tag="yy")
                nc.vector.tensor_tensor(
                    out=yy[:], in0=r[:], in1=r[:], op=ALU.mult)
                nc.vector.tensor_tensor(
                    out=yy[:], in0=yy[:], in1=v[:], op=ALU.mult)
                nc.vector.tensor_scalar(
                    out=yy[:], in0=yy[:], scalar1=-0.5, scalar2=1.5,
                    op0=ALU.mult, op1=ALU.add)
                rn = pstat.tile([128, nblk], f32, tag="rn")
                nc.vector.tensor_tensor(
                    out=rn[:], in0=r[:], in1=yy[:], op=ALU.mult)

                drt = pstat.tile([128, nblk, 128], f16, tag="drt")
                for blk in range(nblk):
                    nc.vector.tensor_scalar_mul(
                        out=drt[:, blk, :], in0=ident[:],
                        scalar1=rn[:, blk:blk + 1])
                return (xh, drt)

            zt_prev = None
            pair_keys = [(g, c0) for g in range(gpc)
                         for c0 in range(0, nchunks, cpair)]
            loads0 = [emit_load(pair_keys[0][0], pair_keys[0][1] + j)
                      for j in range(cpair)]
            pending = [emit_stats(pair_keys[0][0], pair_keys[0][1] + j,
                                  loads0[j]) for j in range(cpair)]
            for pidx, (g, c0) in enumerate(pair_keys):
                cs = list(range(c0, min(c0 + cpair, nchunks)))
                ncs = len(cs)
                cur = pending
                nxt_loads = None
                if pidx + 1 < len(pair_keys):
                    g2, c2 = pair_keys[pidx + 1]
                    nxt_loads = [emit_load(g2, c2 + j) for j in range(cpair)]

                # ---- pass1 per chunk: Z[d, t] = X^T diag(r) ----
                zts = []
                for j, c in enumerate(cs):
                    xh, drt = cur[j]
                    zt = pz.tile([128, dsub, PAD + tch], f16, tag="zt")
                    if c == 0:
                        nc.vector.memset(zt[:, :, 0:PAD], 0.0)
                    else:
                        nc.vector.tensor_copy(
                            out=zt[:, :, 0:PAD],
                            in_=zt_prev[:, :, tch:tch + PAD])
                    for si in range(dsub // 2):
                        zp = pp1.tile([128, 2, tch], f32, tag="zp")
                        for sh in range(2):
                            s = 2 * si + sh
                            for blk in range(nblk):
                                nc.tensor.matmul(
                                    zp[:, sh, blk * 128:(blk + 1) * 128],
                                    lhsT=xh[:, blk, s * 128:(s + 1) * 128],
                                    rhs=drt[:, blk, :],
                                    start=True, stop=True)
                        dst = zt[:, 2 * si:2 * si + 2, PAD:PAD + tch]
                        if si < zcopy_act:
                            nc.scalar.copy(out=dst, in_=zp[:])
                        else:
                            nc.vector.tensor_copy(out=dst, in_=zp[:])
                    zt_prev = zt
                    zts.append(zt)

                # next pair's stats go here: after this pair's zcopies
                # (so they can't delay them on DVE), before the conv
                # (whose ~13us hides the stats chain latency)
                if nxt_loads is not None:
                    pending = [emit_stats(g2, c2 + j, nxt_loads[j])
                               for j in range(cpair)]

                # ---- pass2 paired: conv matmuls share ldweights;
                #      silu writes fp16 channel-major, DMA straight out
                for j, c in enumerate(cs):
                    yh = py.tile([128, dsub, tch], f16, tag="yh")
                    for si in range(dsub // 2):
                        yp = pp2.tile([128, 2, tch], f32, tag="yp")
                        for sh in range(2):
                            s = 2 * si + sh
                            for k in range(K):
                                nc.tensor.matmul(
                                    yp[:, sh, :],
                                    lhsT=wsb[:, g, k, s, :],
                                    rhs=zts[j][:, s, k * DIL:k * DIL + tch],
                                    start=(k == 0), stop=(k == K - 1))
                        nc.scalar.activation(
                            out=yh[:, 2 * si:2 * si + 2, :],
                            in_=yp[:], func=AF.Silu)
                        if si == dsub // 4 - 1:
                            nc.gpsimd.dma_start(
                                out=yv[g, c, :, 0:dsub // 2],
                                in_=yh[:, 0:dsub // 2])
                    nc.gpsimd.dma_start(out=yv[g, c, :, dsub // 2:dsub],
                                        in_=yh[:, dsub // 2:dsub])
    nc.compile()
    return nc


def _host_pack(x, norm_weight, conv_weight):
    """Shard inputs across cores; swizzle for contiguous DMA; fold norm
    weight into compact per-channel conv weight columns."""
    dsub = D // 128
    nblk = TCH // 128
    nchunks = L // TCH
    xg = np.ascontiguousarray(x.transpose(0, 2, 1, 3)).reshape(NGROUPS, L, D)
    xg = xg.astype(np.float16)
    # (g, c, p, blk, d) = x[g, c*tch + blk*128 + p, d]
    xsw = np.ascontiguousarray(
        xg.reshape(NGROUPS, nchunks, nblk, 128, D).transpose(0, 1, 3, 2, 4))

    conv_w = conv_weight.reshape(HC, D, K)            # [hc, d, k]
    weff = conv_w * norm_weight[:, :, None]           # [hc, d, k]
    wr = weff.transpose(0, 2, 1).reshape(HC, K, dsub, 128)  # [hc, k, s, p]
    eye = np.eye(128, dtype=np.float32)
    wdiag = (wr[..., None] * eye).astype(np.float16)  # [hc, K, s, p, m]
    idn = np.eye(128, dtype=np.float16)

    in_maps = []
    for i in range(NCORES):
        gs = [i * GPC + j for j in range(GPC)]
        wcore = np.stack([wdiag[g % HC] for g in gs])  # [gpc, K, s, p, m]
        wpm = np.ascontiguousarray(
            wcore.transpose(3, 0, 1, 2, 4))            # [p, gpc, K, s, m]
        in_maps.append({
            "xin": np.ascontiguousarray(xsw[gs[0]:gs[-1] + 1]),
            "wdg": wpm,
            "idn": idn,
        })
    return in_maps


def _host_unpack(results):
    dsub = D // 128
    nchunks = L // TCH
    # yout per core: [gpc, nchunks, 128, dsub, tch] channel-major
    ys = np.concatenate([r["yout"] for r in results], axis=0)
    ys = ys.reshape(B, HC, nchunks, 128, dsub, TCH)
    # [b, hc, c, p, s, t] -> [b, (c t), hc, (s p)]
    y = ys.transpose(0, 2, 5, 1, 4, 3).reshape(B, L, HC, D)
    return np.ascontiguousarray(y.astype(np.float32))


def _get_prog():
    key = (L, GPC, TCH, CPAIR, SQ_ENGINES, ZCOPY_ACT)
    if key not in _prog_cache:
        _prog_cache[key] = build_program()
    return _prog_cache[key]


def kernel(x, norm_weight, conv_weight, _trace=False, _trace_kwargs=None):
    from concourse.bass_utils import run_bass_kernel_spmd

    x = np.asarray(x, dtype=np.float32)
    norm_weight = np.asarray(norm_weight, dtype=np.float32)
    conv_weight = np.asarray(conv_weight, dtype=np.float32)

    nc = _get_prog()
    in_maps = _host_pack(x, norm_weight, conv_weight)
    res = run_bass_kernel_spmd(
        nc, in_maps, list(range(NCORES)),
        trace=_trace, **(_trace_kwargs or {}))
    out = _host_unpack(res.results)
    if _trace:
        return out, res
    return out


# revision 47
# speedup vs baseline: 1.1154x; 1.0029x over previous
"""Trainium2 Bass kernel for nn_EngramShortConv (RMSNorm + depthwise dilated
causal conv1d + silu), 8-core SPMD.

  x: [B=4, L=4096, HC=4, D=1024] fp32 -> y same shape/dtype.

Sharding: 16 independent (b, hc) groups, 2 per NeuronCore, zero communication.

v3:
  - Output written CHANNEL-MAJOR straight from conv/silu PSUM; host does the
    final un-transpose during unpack (deletes the old PE transpose-back pass
    and its PSUM->SBUF copies).
  - All HBM layouts host-swizzled so every DMA is 128 partitions x 8KB
    contiguous (128 descriptors instead of 512-1024 small ones).
  - Conv diag stationaries built on device from compact per-channel weight
    columns (kills a 2MB / 8192-descriptor weight load).
  - Stats (squares -> rsqrt -> diag(r)) run per *chunk* and are emitted one
    pair ahead so the PE never waits on them.

Per core, per 512-token chunk:
  1. stats: x^2 with 1/D folded accumulates to ms per token (engine per
     128-token block set by SQ_ENGINES); r = rsqrt(ms+eps) via bit-trick +
     1 Newton step on DVE (no ACT table swaps).
  2. pass1 (PE): Z[d, t] = X_blk^T @ diag(r) per 128x128 block -- transpose
     to channel-major with the RMSNorm scale folded in. DVE/ACT copy
     PSUM -> SBUF fp16 (two slabs per instruction) with a 6-column halo
     from the previous chunk.
  3. pass2 (PE): depthwise conv as 4 PSUM-accumulated matmuls
     diag(conv_w[k] * norm_w) @ Z[:, t - 6 + 2k].
  4. ACT Silu reads conv PSUM -> fp16 SBUF; DMA out channel-major.

I/O precision: host casts x to fp16 (halves input DMA); device returns fp16
y upcast to fp32 on host. End-to-end scale-relative error ~3e-3.
"""

import sys

if "/opt/trn_rl_repo" not in sys.path:
    sys.path.insert(0, "/opt/trn_rl_repo")

import numpy as np

B, L, HC, D = 4, 4096, 4, 1024
K, DIL = 4, 2
EPS = 1e-5
PAD = (K - 1) * DIL  # 6
NCORES = 8
NGROUPS = B * HC     # 16
GPC = NGROUPS // NCORES  # 2 groups per core

# tunables
TCH = 512            # tokens per chunk (= matmul moving free dim)
CPAIR = 2            # chunks per conv pairing (shares conv ldweights)
SQ_ENGINES = ("vector", "act", "vector", "vector")  # even chunks (odd use a
# different DVE/ACT interleave so consecutive chunks' stats can't collide on
# one engine queue; see odd_engines below)
ZCOPY_ACT = 2        # of 4 two-slab zcopy units per chunk, how many on ACT

_prog_cache = {}


def build_program(L_=L, gpc=GPC, tch=TCH, cpair=CPAIR,
                  sq_engines=SQ_ENGINES, zcopy_act=ZCOPY_ACT):
    """Build the per-core Bacc program. Same program on all cores (SPMD)."""
    import concourse.bacc as bacc
    import concourse.tile as tile
    from concourse import mybir

    f32 = mybir.dt.float32
    f16 = mybir.dt.float16
    i32 = mybir.dt.int32
    AF = mybir.ActivationFunctionType
    ALU = mybir.AluOpType

    nblk = tch // 128
    dsub = D // 128
    nchunks = L_ // tch
    assert tch % 128 == 0 and L_ % tch == 0 and D % 128 == 0

    nc = bacc.Bacc()
    # host-swizzled input: (g, c, p, blk, d) = x[g, c*tch + blk*128 + p, d]
    xin = nc.declare_dram_parameter("xin", [gpc, nchunks, 128, nblk, D], f16,
                                    isOutput=False)
    # host-built diag stationaries, partition-major so the DMA is 128
    # contiguous 16KB rows: (p, g, k, s, m) = diag(w_eff)[g, k, s][p, m]
    wdg = nc.declare_dram_parameter("wdg", [128, gpc, K, dsub, 128], f16,
                                    isOutput=False)
    idn = nc.declare_dram_parameter("idn", [128, 128], f16, isOutput=False)
    # channel-major output: (g, c, p, s, t) = y[g, d=s*128+p, l=c*tch+t]
    yout = nc.declare_dram_parameter("yout", [gpc, nchunks, 128, dsub, tch],
                                     f16, isOutput=True)

    xv = xin[:]
    yv = yout[:]

    with tile.TileContext(nc) as tc:
        with (
            tc.tile_pool(name="pconst", bufs=1) as pconst,
            tc.tile_pool(name="px", bufs=10) as px,
            tc.tile_pool(name="pstat", bufs=4) as pstat,
            tc.tile_pool(name="pz", bufs=5) as pz,
            tc.tile_pool(name="py", bufs=4) as py,
            tc.tile_pool(name="pp1", bufs=2, space="PSUM") as pp1,
            tc.tile_pool(name="pp2", bufs=2, space="PSUM") as pp2,
        ):
            ident = pconst.tile([128, 128], f16)
            nc.scalar.dma_start(out=ident[:], in_=idn[:])
            wsb = pconst.tile([128, gpc, K, dsub, 128], f16)
            nc.scalar.dma_start(out=wsb[:], in_=wdg[:])

            def emit_load(g, c):
                """Chunk DMA, emitted early to keep prefetch depth."""
                xh = px.tile([128, nblk, D], f16, tag="xh")
                nc.sync.dma_start(out=xh[:], in_=xv[g, c])
                return xh

            def emit_stats(g, c, xh):
                """Stats + r + drt for one chunk. Emitted AFTER the previous
                pair's zcopies so those never queue behind these DVE ops."""
                ssq = pstat.tile([128, nblk], f32, tag="ssq")
                odd_engines = ("act", "vector", "vector", "act")
                for blk in range(nblk):
                    eng = (sq_engines if c % 2 == 0
                           else odd_engines)[blk % len(sq_engines)]
                    scr = pstat.tile([128, D], f16, tag="scr")
                    if eng == "act":
                        nc.scalar.activation(
                            out=scr[:], in_=xh[:, blk, :],
                            func=AF.Square, scale=float(D) ** -0.5,
                            accum_out=ssq[:, blk:blk + 1])
                    else:
                        nc.vector.scalar_tensor_tensor(
                            out=scr[:], in0=xh[:, blk, :],
                            scalar=1.0 / D, in1=xh[:, blk, :],
                            op0=ALU.mult, op1=ALU.mult,
                            accum_out=ssq[:, blk:blk + 1])
                # r = rsqrt(ms+eps): bit trick + 1 Newton step on
                # DVE (keeps Sqrt out of ACT -> zero table swaps)
                v = pstat.tile([128, nblk], f32, tag="v")
                nc.vector.tensor_scalar(
                    out=v[:], in0=ssq[:], scalar1=EPS, scalar2=None,
                    op0=ALU.add)
                r = pstat.tile([128, nblk], f32, tag="r")
                nc.vector.tensor_scalar(
                    out=r[:].bitcast(i32), in0=v[:].bitcast(i32),
                    scalar1=1, scalar2=None, op0=ALU.arith_shift_right)
                nc.vector.tensor_scalar(
                    out=r[:].bitcast(i32), in0=r[:].bitcast(i32),
                    scalar1=-1, scalar2=0x5F3759DF,
                    op0=ALU.mult, op1=ALU.add)
                yy = pstat.tile([128, nblk], f32, tag="yy")
                nc.vector.tensor_tensor(
                    out=yy[:], in0=r[:], in1=r[:], op=ALU.mult)
                nc.vector.tensor_tensor(
                    out=yy[:], in0=yy[:], in1=v[:], op=ALU.mult)
                nc.vector.tensor_scalar(
                    out=yy[:], in0=yy[:], scalar1=-0.5, scalar2=1.5,
                    op0=ALU.mult, op1=ALU.add)
                rn = pstat.tile([128, nblk], f32, tag="rn")
                nc.vector.tensor_tensor(
                    out=rn[:], in0=r[:], in1=yy[:], op=ALU.mult)

                drt = pstat.tile([128, nblk, 128], f16, tag="drt")
                for blk in range(nblk):
                    nc.vector.tensor_scalar_mul(
                        out=drt[:, blk, :], in0=ident[:],
                        scalar1=rn[:, blk:blk + 1])
                return (xh, drt)

            zt_prev = None
            pair_keys = [(g, c0) for g in range(gpc)
                         for c0 in range(0, nchunks, cpair)]
            loads0 = [emit_load(pair_keys[0][0], pair_keys[0][1] + j)
                      for j in range(cpair)]
            pending = [emit_stats(pair_keys[0][0], pair_keys[0][1] + j,
                                  loads0[j]) for j in range(cpair)]
            for pidx, (g, c0) in enumerate(pair_keys):
                cs = list(range(c0, min(c0 + cpair, nchunks)))
                ncs = len(cs)
                cur = pending
                nxt_loads = None
                if pidx + 1 < len(pair_keys):
                    g2, c2 = pair_keys[pidx + 1]
                    nxt_loads = [emit_load(g2, c2 + j) for j in range(cpair)]

                # ---- pass1 per chunk: Z[d, t] = X^T diag(r) ----
                zts = []
                for j, c in enumerate(cs):
                    xh, drt = cur[j]
                    zt = pz.tile([128, dsub, PAD + tch], f16, tag="zt")
                    if c == 0:
                        nc.vector.memset(zt[:, :, 0:PAD], 0.0)
                    else:
                        nc.vector.tensor_copy(
                            out=zt[:, :, 0:PAD],
                            in_=zt_prev[:, :, tch:tch + PAD])
                    for si in range(dsub // 2):
                        zp = pp1.tile([128, 2, tch], f32, tag="zp")
                        for sh in range(2):
                            s = 2 * si + sh
                            for blk in range(nblk):
                                nc.tensor.matmul(
                                    zp[:, sh, blk * 128:(blk + 1) * 128],
                                    lhsT=xh[:, blk, s * 128:(s + 1) * 128],
                                    rhs=drt[:, blk, :],
                                    start=True, stop=True)
                        dst = zt[:, 2 * si:2 * si + 2, PAD:PAD + tch]
                        if si < zcopy_act:
                            nc.scalar.copy(out=dst, in_=zp[:])
                        else:
                            nc.vector.tensor_copy(out=dst, in_=zp[:])
                    zt_prev = zt
                    zts.append(zt)

                # next pair's stats go here: after this pair's zcopies
                # (so they can't delay them on DVE), before the conv
                # (whose ~13us hides the stats chain latency)
                if nxt_loads is not None:
                    pending = [emit_stats(g2, c2 + j, nxt_loads[j])
                               for j in range(cpair)]

                # ---- pass2 paired: conv matmuls share ldweights;
                #      silu writes fp16 channel-major, DMA straight out
                for j, c in enumerate(cs):
                    yh = py.tile([128, dsub, tch], f16, tag="yh")
                    for si in range(dsub // 2):
                        yp = pp2.tile([128, 2, tch], f32, tag="yp")
                        for sh in range(2):
                            s = 2 * si + sh
                            for k in range(K):
                                nc.tensor.matmul(
                                    yp[:, sh, :],
                                    lhsT=wsb[:, g, k, s, :],
                                    rhs=zts[j][:, s, k * DIL:k * DIL + tch],
                                    start=(k == 0), stop=(k == K - 1))
                        nc.scalar.activation(
                            out=yh[:, 2 * si:2 * si + 2, :],
                            in_=yp[:], func=AF.Silu)
                        if si == dsub // 4 - 1:
                            nc.gpsimd.dma_start(
                                out=yv[g, c, :, 0:dsub // 2],
                                in_=yh[:, 0:dsub // 2])
                    nc.gpsimd.dma_start(out=yv[g, c, :, dsub // 2:dsub],
                                        in_=yh[:, dsub // 2:dsub])
    nc.compile()
    return nc


def _host_pack(x, norm_weight, conv_weight):
    """Shard inputs across cores; swizzle for contiguous DMA; fold norm
    weight into compact per-channel conv weight columns."""
    dsub = D // 128
    nblk = TCH // 128
    nchunks = L // TCH
    xg = np.ascontiguousarray(x.transpose(0, 2, 1, 3)).reshape(NGROUPS, L, D)
    xg = xg.astype(np.float16)
    # (g, c, p, blk, d) = x[g, c*tch + blk*128 + p, d]
    xsw = np.ascontiguousarray(
        xg.reshape(NGROUPS, nchunks, nblk, 128, D).transpose(0, 1, 3, 2, 4))

    conv_w = conv_weight.reshape(HC, D, K)            # [hc, d, k]
    weff = conv_w * norm_weight[:, :, None]           # [hc, d, k]
    wr = weff.transpose(0, 2, 1).reshape(HC, K, dsub, 128)  # [hc, k, s, p]
    eye = np.eye(128, dtype=np.float32)
    wdiag = (wr[..., None] * eye).astype(np.float16)  # [hc, K, s, p, m]
    idn = np.eye(128, dtype=np.float16)

    in_maps = []
    for i in range(NCORES):
        gs = [i * GPC + j for j in range(GPC)]
        wcore = np.stack([wdiag[g % HC] for g in gs])  # [gpc, K, s, p, m]
        wpm = np.ascontiguousarray(
            wcore.transpose(3, 0, 1, 2, 4))            # [p, gpc, K, s, m]
        in_maps.append({
            "xin": np.ascontiguousarray(xsw[gs[0]:gs[-1] + 1]),
            "wdg": wpm,
            "idn": idn,
        })
    return in_maps


def _host_unpack(results):
    dsub = D // 128
    nchunks = L // TCH
    # yout per core: [gpc, nchunks, 128, dsub, tch] channel-major
    ys = np.concatenate([r["yout"] for r in results], axis=0)
    ys = ys.reshape(B, HC, nchunks, 128, dsub, TCH)
    # [b, hc, c, p, s, t] -> [b, (c t), hc, (s p)]
    y = ys.transpose(0, 2, 5, 1, 4, 3).reshape(B, L, HC, D)
    return np.ascontiguousarray(y.astype(np.float32))


def _get_prog():
    key = (L, GPC, TCH, CPAIR, SQ_ENGINES, ZCOPY_ACT)
    if key not in _prog_cache:
        _prog_cache[key] = build_program()
    return _prog_cache[key]


def kernel(x, norm_weight, conv_weight, _trace=False, _trace_kwargs=None):
    from concourse.bass_utils import run_bass_kernel_spmd

    x = np.asarray(x, dtype=np.float32)
    norm_weight = np.asarray(norm_weight, dtype=np.float32)
    conv_weight = np.asarray(conv_weight, dtype=np.float32)

    nc = _get_prog()
    in_maps = _host_pack(x, norm_weight, conv_weight)
    res = run_bass_kernel_spmd(
        nc, in_maps, list(range(NCORES)),
        trace=_trace, **(_trace_kwargs or {}))
    out = _host_unpack(res.results)
    if _trace:
        return out, res
    return out
